# revision 1
# baseline (speedup 1.0000x reference)
"""Trainium2 Bass kernel for causal multi-head attention with RoPE + register tokens.

Problem (nn_Attention_38293928411140):
  B=1, S=4096, HIDDEN=512, 8 heads x head_dim 64, causal SDPA, RoPE applied to
  positions >= num_registers (cos/sin indexed by position - num_registers), fp32.
  out = softmax(causal(QK^T/8)) V followed by a Wo projection.

Sharding: tensor-parallel over heads -- one head per NeuronCore (8 heads, 8 cores).

Per-core kernel (fused causal pipeline; all heavy matmuls in float32r, the PE's
1-cycle/column fast-fp32 mode -- measured output error ~2.5e-4 relative; set
fast_mm=False for bit-accurate fp32 at ~3x the PE cost):
  - X^T built on-chip per 512-column chunk (PE transpose-mode matmuls)
  - Q^T/K^T projections (stacked [Wq|Wk], plus a rotate_half-premultiplied
    weight set so RoPE becomes pure elementwise mul/add on DVE+GPSIMD) and V
  - causal flash attention in transposed orientation: scores^T [k, q] chunks on
    PE, exp on ScalarE reading PSUM in 1024-wide strides (softmax max-shift
    skipped -- exact by shift invariance, scores are bounded), causal mask via
    gpsimd affine_select after exp, O^T accumulated in PSUM with a ones-column
    appended to V producing softmax row-sums for free
  - supertile c of attention only needs chunks <= c, so chunk prep for c+1 is
    emitted interleaved with attention supertile c and overlaps it across
    engines (PE/ACT/DVE/GPSIMD/DMA all concurrently busy)
  - per-supertile normalization (K=1 ones-broadcast matmul + DVE) feeds the
    AllToAll head-exchange; each core then owns all 8 heads for its 512-row
    s-shard and computes that slice of the final Wo projection.
Host side only packs weights (per-head slices, transposes, rotate_half fold,
identity matrix, full-length cos/sin tables with identity rotation for register
tokens) and concatenates the 8 output shards.

A post-scheduling pass hoists extra semaphore waits onto sequencer no-ops
because this walrus build rejects instructions with more than one sync wait.
"""
import math
import numpy as np

import concourse.bass as bass
import concourse.mybir as mybir
import concourse.tile as tile
from concourse import library_config
from concourse.bass import _add_dep_helper
from concourse.bass_utils import run_bass_kernel_spmd

F32 = mybir.dt.float32
F32R = mybir.dt.float32r

HIDDEN = 512
NHEADS = 8
HD = 64
NCORES = 8
SCALE = 1.0 / math.sqrt(HD)

_PROGRAM_CACHE = {}


_HOIST_TYPES = {"InstMatmult", "InstDrain", "InstDMACopy"}


def _split_matmul_waits(nc):
    """Walrus's CoreV3 codegen rejects instructions carrying more than one sync
    wait ('Too many sync wait commands', e.g. Matmult LW_STRUCT and Drain).
    Hoist all but one wait onto same-engine sequencer no-ops inserted right
    before the instruction -- semantically identical (the sequencer satisfies
    the waits in program order before issuing it)."""
    import bass_rust
    for f in nc.m.functions:
        for blk in f.blocks:
            out = []
            for inst in blk.instructions:
                si = getattr(inst, "sync_info", None)
                eng = getattr(inst, "engine", None)
                if si is not None and eng is not None and len(si.on_wait) > 1:
                    waits = list(si.on_wait)
                    for k, w in enumerate(waits[:-1]):
                        nop = bass_rust.InstNoOp(
                            name=f"{inst.name}-hw{k}",
                            engine=eng,
                            text_hint="hoisted-wait",
                            sync_info=mybir.SyncInfo(on_wait=[w], on_update=[]),
                        )
                        out.append(nop)
                    inst.sync_info = mybir.SyncInfo(
                        on_wait=[waits[-1]], on_update=list(si.on_update))
                out.append(inst)
            blk.instructions = out


def build_program(S=4096, hoist=True, repeat=1, mock_cc=False, hw_loop=0,
                  fast_mm=True):
    """Build the SPMD Bass program (same NEFF on all 8 cores).

    Fused causal pipeline: supertile c of the attention only needs Q/K/V
    chunks <= c, so chunk prep (X-transpose, projections, rope, V layout) for
    chunk c+1 is emitted interleaved with attention supertile c and overlaps
    it across engines."""
    assert S % 512 == 0
    W = 512                      # q-supertile width == s-chunk width
    NSUP = S // W
    NST = S // 128
    SHARD = S // NCORES

    nc = bass.Bass("TRN2", target_bir_lowering=False, debug=False,
                   num_devices=NCORES)

    x = nc.dram_tensor("x", [S, HIDDEN], F32R, kind="ExternalInput").ap()
    wqkT = nc.dram_tensor("wqkT", [HIDDEN, 2 * HD], F32R, kind="ExternalInput").ap()
    wqkrotT = nc.dram_tensor("wqkrotT", [HIDDEN, 2 * HD], F32R, kind="ExternalInput").ap()
    wvT = nc.dram_tensor("wvT", [HIDDEN, HD], F32R, kind="ExternalInput").ap()
    woT = nc.dram_tensor("woT", [HIDDEN, HIDDEN], F32R, kind="ExternalInput").ap()
    cosT = nc.dram_tensor("cosT", [HD, S], F32, kind="ExternalInput").ap()
    sinT = nc.dram_tensor("sinT", [HD, S], F32, kind="ExternalInput").ap()
    out_shard = nc.dram_tensor("out_shard", [SHARD, HIDDEN], F32,
                               kind="ExternalOutput").ap()

    ident_in = nc.dram_tensor("ident", [128, 128], F32R, kind="ExternalInput").ap()
    a2a_in = nc.dram_tensor("a2a_in", [NCORES, HD, SHARD], F32R)
    a2a_out = nc.dram_tensor("a2a_out", [NCORES, HD, SHARD], F32R)

    Exp = mybir.ActivationFunctionType.Exp
    R = (lambda ap: ap.bitcast(F32R)) if fast_mm else (lambda ap: ap)

    with tile.TileContext(nc) as tc:
      with tc.tile_pool(name="persist", bufs=1) as pp:
        ident = pp.tile([128, 128], F32R)
        qt = pp.tile([64, S], F32R, tag="qt")        # roped Q^T [d, s]
        kt = pp.tile([64, S], F32R, tag="kt")        # roped K^T [d, s]
        vext = pp.tile([128, NST * 65], F32R, tag="vext")  # V tiles + ones col
        ones_row = pp.tile([1, 64], F32, tag="ones")
        ones_f32 = pp.tile([128, 32], F32, tag="ones32")
        wqk_sb = pp.tile([128, 4 * 128], F32R, tag="wqk")
        wrot_sb = pp.tile([128, 4 * 128], F32R, tag="wrot")
        wv_sb = pp.tile([128, 4 * 64], F32R, tag="wv")
        cos_sb = pp.tile([64, S], F32, tag="cos")
        sin_sb = pp.tile([64, S], F32, tag="sin")
        wo_sb = pp.tile([128, 4 * 512], F32R, tag="wo")

        nc.scalar.dma_start(ident[:], ident_in)
        nc.gpsimd.memset(ones_f32[:], 1.0)
        nc.scalar.copy(
            vext[:].rearrange("p (t c) -> p t c", c=65)[:, :, 64],
            ones_f32[:, 0:NST])
        nc.gpsimd.memset(ones_row[:], 1.0)
        for hj in range(4):
            nc.scalar.dma_start(wqk_sb[:, hj * 128:(hj + 1) * 128],
                                wqkT[hj * 128:(hj + 1) * 128, :])
            nc.scalar.dma_start(wrot_sb[:, hj * 128:(hj + 1) * 128],
                                wqkrotT[hj * 128:(hj + 1) * 128, :])
            nc.scalar.dma_start(wv_sb[:, hj * 64:(hj + 1) * 64],
                                wvT[hj * 128:(hj + 1) * 128, :])
        nc.scalar.dma_start(cos_sb[:], cosT)
        nc.scalar.dma_start(sin_sb[:], sinT)
        for hj in range(4):
            nc.scalar.dma_start(wo_sb[:, hj * 512:(hj + 1) * 512],
                                woT[hj * 128:(hj + 1) * 128, :])

        import contextlib
        loop_cm = tc.For_i(0, hw_loop, 1) if hw_loop else contextlib.nullcontext()
        with loop_cm:
          for _rep in range(repeat):
            with tc.tile_pool(name="attn", bufs=1) as pa, \
                 tc.tile_pool(name="xin", bufs=3) as pxin, \
                 tc.tile_pool(name="xtc", bufs=2) as pxtc, \
                 tc.tile_pool(name="vtc", bufs=2) as pvtc, \
                 tc.tile_pool(name="rope", bufs=2) as prt, \
                 tc.tile_pool(name="pt", bufs=3) as ppt, \
                 tc.tile_pool(name="psc", bufs=2, space="PSUM") as psc, \
                 tc.tile_pool(name="pprep", bufs=3, space="PSUM") as pprep, \
                 tc.tile_pool(name="psot", bufs=1, space="PSUM") as psot:
                ot = pa.tile([64, S], F32R, tag="ot")
                rowsum = pa.tile([1, S], F32, tag="rowsum")

                xtc_by_c = {}

                def prep_a(c):
                    xg = pxin.tile([128, 4, HIDDEN], F32R, tag="xin", name="xg")
                    nc.sync.dma_start(
                        xg[:],
                        x[c * 512:(c + 1) * 512, :].rearrange(
                            "(k p) h -> p k h", p=128))
                    xtc = []
                    for hj in range(4):
                        ps = pprep.tile([128, 512], F32, tag="prep", name="trp")
                        for k in range(4):
                            nc.tensor.transpose(
                                R(ps[:, k * 128:(k + 1) * 128]),
                                R(xg[:, k, hj * 128:(hj + 1) * 128]),
                                R(ident[:]))
                        xtile = pxtc.tile([128, 512], F32R, tag=f"xt{hj}",
                                          name=f"xt{hj}")
                        nc.vector.tensor_copy(xtile[:], ps[:])
                        xtc.append(xtile)
                    xtc_by_c[c] = xtc

                def prep_b1(c):
                    cs = slice(c * 512, (c + 1) * 512)
                    xtc = xtc_by_c[c]
                    pqk = pprep.tile([128, 512], F32, tag="prep", name="pqk")
                    for hj in range(4):
                        nc.tensor.matmul(
                            pqk[:], lhsT=R(wqk_sb[:, hj * 128:(hj + 1) * 128]),
                            rhs=R(xtc[hj][:]), start=(hj == 0), stop=(hj == 3))
                    prot = pprep.tile([128, 512], F32, tag="prep", name="prot")
                    for hj in range(4):
                        nc.tensor.matmul(
                            prot[:], lhsT=R(wrot_sb[:, hj * 128:(hj + 1) * 128]),
                            rhs=R(xtc[hj][:]), start=(hj == 0), stop=(hj == 3))
                    t1 = prt.tile([128, 512], F32, tag="t1", name="t1")
                    t2 = prt.tile([128, 512], F32, tag="t2", name="t2")
                    nc.vector.tensor_mul(t1[0:64, :], pqk[0:64, :], cos_sb[:, cs])
                    nc.vector.tensor_mul(t1[64:128, :], pqk[64:128, :], cos_sb[:, cs])
                    nc.vector.tensor_mul(t2[0:64, :], prot[0:64, :], sin_sb[:, cs])
                    nc.vector.tensor_mul(t2[64:128, :], prot[64:128, :], sin_sb[:, cs])
                    nc.gpsimd.tensor_add(qt[:, cs], t1[0:64, :], t2[0:64, :])
                    nc.gpsimd.tensor_add(kt[:, cs], t1[64:128, :], t2[64:128, :])

                def prep_b2(c):
                    cs = slice(c * 512, (c + 1) * 512)
                    xtc = xtc_by_c.pop(c)
                    pv = pprep.tile([64, 512], F32, tag="prep", name="pv")
                    for hj in range(4):
                        nc.tensor.matmul(
                            pv[:], lhsT=R(wv_sb[:, hj * 64:(hj + 1) * 64]),
                            rhs=R(xtc[hj][:]), start=(hj == 0), stop=(hj == 3))
                    vtc = pvtc.tile([64, 512], F32R, tag="vtc", name="vtc")
                    nc.vector.tensor_copy(vtc[:], pv[:])
                    pst = pprep.tile([128, 256], F32, tag="prep", name="pst")
                    for k in range(4):
                        nc.tensor.transpose(
                            R(pst[:, k * 64:(k + 1) * 64]),
                            R(vtc[:, k * 128:(k + 1) * 128]),
                            R(ident[0:64, 0:64]))
                    nc.vector.tensor_copy(
                        vext[:].rearrange("p (t c) -> p t c", c=65)[
                            :, 4 * c:4 * c + 4, 0:64],
                        pst[:].rearrange("p (t c) -> p t c", c=64))

                def attn_groups(sup, otp, glo, ghi):
                    qs = slice(sup * 512, (sup + 1) * 512)
                    npairs = (sup + 1) * 4
                    for g in range(glo, ghi):
                        pg = min(2, npairs - g * 2)
                        sp = psc.tile([128, pg * 512], F32, tag="sc", name="sp")
                        for p in range(pg):
                            kp = g * 2 + p
                            o = p * 512
                            nc.tensor.matmul(
                                sp[:, o:o + 512],
                                lhsT=R(kt[:, kp * 128:(kp + 1) * 128]),
                                rhs=R(qt[:, qs]), start=True, stop=True)
                        ptile = ppt.tile([128, pg * 512], F32R, tag="pt",
                                         name="ptile")
                        nc.scalar.activation(ptile[:, 0:pg * 512],
                                             sp[:, 0:pg * 512], Exp, scale=SCALE)
                        for p in range(pg):
                            kp = g * 2 + p
                            if kp >= sup * 4:
                                nc.gpsimd.affine_select(
                                    out=ptile[:, p * 512:(p + 1) * 512],
                                    in_=ptile[:, p * 512:(p + 1) * 512],
                                    pattern=[[1, 512]],
                                    compare_op=mybir.AluOpType.is_ge, fill=0.0,
                                    base=sup * 512 - kp * 128,
                                    channel_multiplier=-1)
                        for p in range(pg):
                            kp = g * 2 + p
                            nc.tensor.matmul(
                                otp[:], lhsT=R(vext[:, kp * 65:kp * 65 + 65]),
                                rhs=R(ptile[:, p * 512:(p + 1) * 512]),
                                start=(kp == 0), stop=(kp == npairs - 1))
                def attn_tail(sup, otp):
                    qs = slice(sup * 512, (sup + 1) * 512)
                    nc.vector.reciprocal(rowsum[0:1, qs], otp[64:65, :])
                    rb = pprep.tile([64, 512], F32, tag="prep", name="rb")
                    nc.tensor.matmul(rb[:], lhsT=ones_row[:],
                                     rhs=rowsum[0:1, qs], start=True, stop=True)
                    rbs = prt.tile([64, 512], F32, tag="rbs", name="rbs")
                    nc.vector.tensor_copy(rbs[:], rb[:])
                    nc.vector.tensor_mul(ot[:, qs], otp[0:64, :], rbs[:])
                    for j in range(sup * W // SHARD, (sup + 1) * W // SHARD):
                        nc.sync.dma_start(a2a_in.ap()[j],
                                          ot[:, j * SHARD:(j + 1) * SHARD])

                prep_a(0)
                prep_b1(0)
                prep_b2(0)
                prep_a(1)
                prep_b1(1)
                prep_b2(1)
                for sup in range(NSUP):
                    ngroups = ((sup + 1) * 4 + 1) // 2
                    otp = psot.tile([65, 512], F32, tag="otp", name="otp")
                    nxt = sup + 2
                    if nxt < NSUP:
                        attn_groups(sup, otp, 0, max(1, ngroups // 2))
                        prep_a(nxt)
                        attn_groups(sup, otp, max(1, ngroups // 2), ngroups)
                        prep_b1(nxt)
                        prep_b2(nxt)
                    else:
                        attn_groups(sup, otp, 0, ngroups)
                    attn_tail(sup, otp)

                # ---- exchange heads, output projection ----
                with tc.tile_pool(name="fin", bufs=1) as pf, \
                     tc.tile_pool(name="fout", bufs=2) as pfo:
                    if mock_cc:
                        nc.gpsimd.dma_start(a2a_out.ap(), a2a_in.ap())
                    else:
                        nc.gpsimd.collective_compute(
                            "AllToAll", mybir.AluOpType.bypass,
                            replica_groups=[list(range(NCORES))],
                            ins=[a2a_in.ap()], outs=[a2a_out.ap()])
                    otall = pf.tile([128, 4 * SHARD], F32R, tag="otall")
                    a2a_flat = a2a_out.ap().rearrange(
                        "e d s -> (e d) s").rearrange("(c p) s -> c p s", p=128)
                    for ci in range(4):
                        nc.sync.dma_start(
                            otall[:, ci * SHARD:(ci + 1) * SHARD], a2a_flat[ci])
                    for ss in range(SHARD // 128):
                        po = psc.tile([128, 512], F32, tag="sc", name="po")
                        for ci in range(4):
                            nc.tensor.matmul(
                                po[:],
                                lhsT=R(otall[:, ci * SHARD + ss * 128:
                                             ci * SHARD + (ss + 1) * 128]),
                                rhs=R(wo_sb[:, ci * 512:(ci + 1) * 512]),
                                start=(ci == 0), stop=(ci == 3))
                        osb = pfo.tile([128, 512], F32, tag="osb", name="osb")
                        nc.scalar.copy(osb[:], po[:])
                        nc.sync.dma_start(
                            out_shard[ss * 128:(ss + 1) * 128, :], osb[:])
    if hoist:
        _split_matmul_waits(nc)
    return nc


def get_program(S=4096):
    if S not in _PROGRAM_CACHE:
        _PROGRAM_CACHE[S] = build_program(S)
    return _PROGRAM_CACHE[S]


def make_in_maps(hidden_states, Wq, Wk, Wv, Wo, cos, sin, num_registers, S):
    """Host-side packing: slice per-head weights, fold rotate_half into weights,
    build full-length transposed cos/sin tables (identity rotation for the
    register tokens)."""
    nr = int(num_registers)
    X = np.ascontiguousarray(np.asarray(hidden_states, dtype=np.float32).reshape(S, HIDDEN))
    Wq = np.asarray(Wq, dtype=np.float32)
    Wk = np.asarray(Wk, dtype=np.float32)
    Wv = np.asarray(Wv, dtype=np.float32)
    Wo = np.asarray(Wo, dtype=np.float32)
    cos = np.asarray(cos, dtype=np.float32)
    sin = np.asarray(sin, dtype=np.float32)

    cos_full = np.ones((S, HD), np.float32)
    sin_full = np.zeros((S, HD), np.float32)
    if nr < S:
        cos_full[nr:] = cos[:S - nr]
        sin_full[nr:] = sin[:S - nr]
    cosT = np.ascontiguousarray(cos_full.T)
    sinT = np.ascontiguousarray(sin_full.T)
    woT = np.ascontiguousarray(Wo.T)

    def rot(Wh):
        return np.concatenate([-Wh[HD // 2:], Wh[:HD // 2]], axis=0)

    in_maps = []
    for c in range(NCORES):
        sl = slice(c * HD, (c + 1) * HD)
        Wq_h, Wk_h, Wv_h = Wq[sl], Wk[sl], Wv[sl]
        wqkT = np.ascontiguousarray(np.concatenate([Wq_h, Wk_h], axis=0).T)
        wqkrotT = np.ascontiguousarray(
            np.concatenate([rot(Wq_h), rot(Wk_h)], axis=0).T)
        wvT = np.ascontiguousarray(Wv_h.T)
        in_maps.append({
            "x": X, "wqkT": wqkT, "wqkrotT": wqkrotT, "wvT": wvT,
            "woT": woT, "cosT": cosT, "sinT": sinT,
            "ident": np.eye(128, dtype=np.float32),
        })
    return in_maps


def kernel(hidden_states, Wq, Wk, Wv, Wo, cos, sin, num_registers):
    hidden_states = np.asarray(hidden_states)
    B, S, H = hidden_states.shape
    assert B == 1 and H == HIDDEN
    nc = get_program(S)
    in_maps = make_in_maps(hidden_states, Wq, Wk, Wv, Wo, cos, sin,
                           num_registers, S)
    res = run_bass_kernel_spmd(nc, in_maps, list(range(NCORES)))
    shards = [res.results[c]["out_shard"] for c in range(NCORES)]
    out = np.concatenate(shards, axis=0).reshape(1, S, HIDDEN)
    return out.astype(np.float32)



# revision 3
# speedup vs baseline: 1.6955x; 1.6955x over previous
"""Trainium2 Bass kernel for causal multi-head attention with RoPE + register tokens.

Problem (nn_Attention_38293928411140):
  B=1, S=4096, HIDDEN=512, 8 heads x head_dim 64, causal SDPA, RoPE applied to
  positions >= num_registers (cos/sin indexed by position - num_registers), fp32.
  out = softmax(causal(QK^T/8)) V followed by a Wo projection.

Sharding: tensor-parallel over heads -- one head per NeuronCore, no collective.
Each core emits an UNNORMALIZED per-head partial of the output projection
(partialT = Wo_h^T . (exp-scores . V)^T, [512, S]) plus the softmax row-sums;
the host divides by the row-sums and adds the 8 partials.

Per-core kernel, all matmuls bf16 (inputs pre-rounded host-side; tolerance is
2e-2 and measured error is ~2e-3):
  - X^T is transposed on the HOST and streamed in bf16, so no PE transposes.
  - Q^T/K^T projection in one [128,512] psum block per 512-chunk; RoPE applied
    on DVE: rotate_half is a partition shuffle (sign folded into the host-built
    sinneg table), all ops bf16 SBUF->SBUF at 4x DVE rate.
  - V projected directly in natural [s, d] orientation (bf16 matmuls are
    1 cycle/row at any width) -- no V transpose; a ones-column appended to V
    makes the attention matmul produce softmax row-sums for free.
  - causal flash attention in transposed orientation: scores^T [k, q] chunks
    on PE, exp on the scalar engine (the only engine with activation hw;
    max-shift skipped -- exact by shift invariance, scores are bounded),
    diagonal chunks compute/exp only the causally live column range and mask
    just the 128-wide boundary block via gpsimd affine_select after exp.
  - per-supertile tail: psum -> bf16 copy (row 64 = row-sums), 4 output-
    projection matmuls (contract dim 64), psum -> bf16 copies, DMA out.
  - chunk prep for c+1 is emitted interleaved with attention supertile c and
    overlaps it across engines (PE/ACT/DVE/Pool/DMA all concurrently busy).

A post-scheduling pass hoists extra semaphore waits onto sequencer no-ops
because this walrus build rejects instructions with more than one sync wait.
"""
import math
import numpy as np
import ml_dtypes

import concourse.bass as bass
import concourse.mybir as mybir
import concourse.tile as tile

from concourse.bass_utils import run_bass_kernel_spmd

F32 = mybir.dt.float32
BF16 = mybir.dt.bfloat16

HIDDEN = 512
NHEADS = 8
HD = 64
NCORES = 8
SCALE = 1.0 / math.sqrt(HD)

_PROGRAM_CACHE = {}

_HOIST_TYPES = {"InstMatmult", "InstDrain", "InstDMACopy"}


def _split_matmul_waits(nc):
    """Walrus's CoreV3 codegen rejects instructions carrying more than one sync
    wait ('Too many sync wait commands', e.g. Matmult LW_STRUCT and Drain).
    Hoist all but one wait onto same-engine sequencer no-ops inserted right
    before the instruction -- semantically identical (the sequencer satisfies
    the waits in program order before issuing it)."""
    import bass_rust
    for f in nc.m.functions:
        for blk in f.blocks:
            out = []
            for inst in blk.instructions:
                si = getattr(inst, "sync_info", None)
                eng = getattr(inst, "engine", None)
                if si is not None and eng is not None and len(si.on_wait) > 1:
                    waits = list(si.on_wait)
                    for k, w in enumerate(waits[:-1]):
                        nop = bass_rust.InstNoOp(
                            name=f"{inst.name}-hw{k}",
                            engine=eng,
                            text_hint="hoisted-wait",
                            sync_info=mybir.SyncInfo(on_wait=[w], on_update=[]),
                        )
                        out.append(nop)
                    inst.sync_info = mybir.SyncInfo(
                        on_wait=[waits[-1]], on_update=list(si.on_update))
                out.append(inst)
            blk.instructions = out


def build_program(S=4096, hoist=True, repeat=1, mock_cc=False, hw_loop=0,
                  fast_mm=True):
    """Build the SPMD Bass program (same NEFF on all 8 cores, no collectives).

    Fused causal pipeline: supertile `sup` of the attention only needs Q/K/V
    chunks <= sup, so chunk prep for sup+1 is emitted interleaved with
    attention supertile sup and overlaps it across engines."""
    assert S % 512 == 0
    W = 512                      # q-supertile width == s-chunk width
    NSUP = S // W
    NST = S // 128

    nc = bass.Bass("TRN2", target_bir_lowering=False, debug=False,
                   num_devices=NCORES)

    xT = nc.dram_tensor("xT", [HIDDEN, S], BF16, kind="ExternalInput").ap()
    wqk = nc.dram_tensor("wqk", [HIDDEN, 2 * HD], BF16, kind="ExternalInput").ap()
    wv = nc.dram_tensor("wv", [HIDDEN, HD], BF16, kind="ExternalInput").ap()
    woh = nc.dram_tensor("woh", [HD, HIDDEN], BF16, kind="ExternalInput").ap()
    cosT = nc.dram_tensor("cosT", [HD, S], BF16, kind="ExternalInput").ap()
    sinnegT = nc.dram_tensor("sinnegT", [HD, S], BF16, kind="ExternalInput").ap()
    partialT = nc.dram_tensor("partialT", [HIDDEN, S], BF16,
                              kind="ExternalOutput").ap()
    rowsums = nc.dram_tensor("rowsums", [1, S], BF16,
                             kind="ExternalOutput").ap()

    Exp = mybir.ActivationFunctionType.Exp

    with tile.TileContext(nc) as tc:
      with tc.tile_pool(name="persist", bufs=1) as pp:
        wqk_sb = pp.tile([128, 4, 128], BF16, tag="wqk")
        wv_sb = pp.tile([128, 4, HD], BF16, tag="wv")
        woh_sb = pp.tile([HD, HIDDEN], BF16, tag="woh")
        cos2 = pp.tile([128, S], BF16, tag="cos2")
        sinneg2 = pp.tile([128, S], BF16, tag="sinneg2")
        qt = pp.tile([HD, S], BF16, tag="qt")        # roped Q^T [d, s]
        kt = pp.tile([HD, S], BF16, tag="kt")        # roped K^T [d, s]
        vext = pp.tile([128, NST * 65], BF16, tag="vext")  # V tiles + ones col

        # ones columns of vext (data columns are overwritten per chunk)
        nc.gpsimd.memset(vext[:], 1.0)
        nc.gpsimd.dma_start(wqk_sb[:],
                            wqk.rearrange("(k p) c -> p k c", p=128))
        nc.gpsimd.dma_start(wv_sb[:],
                            wv.rearrange("(k p) c -> p k c", p=128))
        nc.gpsimd.dma_start(woh_sb[:], woh)
        nc.scalar.dma_start(cos2[0:HD, :], cosT)
        nc.scalar.dma_start(cos2[HD:128, :], cosT)
        nc.gpsimd.dma_start(sinneg2[0:HD, :], sinnegT)
        nc.gpsimd.dma_start(sinneg2[HD:128, :], sinnegT)

        import contextlib
        loop_cm = tc.For_i(0, hw_loop, 1) if hw_loop else contextlib.nullcontext()
        with loop_cm:
          for _rep in range(repeat):
            with tc.tile_pool(name="xin", bufs=3) as pxin, \
                 tc.tile_pool(name="qkbf", bufs=2) as pqkbf, \
                 tc.tile_pool(name="rope", bufs=2) as prt, \
                 tc.tile_pool(name="pt", bufs=3) as ppt, \
                 tc.tile_pool(name="otb", bufs=2) as pot, \
                 tc.tile_pool(name="pob", bufs=3) as pposb, \
                 tc.tile_pool(name="psc", bufs=2, space="PSUM") as psc, \
                 tc.tile_pool(name="pprep", bufs=2, space="PSUM") as pprep, \
                 tc.tile_pool(name="psot", bufs=1, space="PSUM") as psot, \
                 tc.tile_pool(name="ppo", bufs=1, space="PSUM") as ppo:

                xtc_by_c = {}

                def prep_a(c):
                    xt = pxin.tile([128, 4, W], BF16, tag="xin", name="xt")
                    nc.sync.dma_start(
                        xt[:],
                        xT[:, c * W:(c + 1) * W].rearrange(
                            "(k p) s -> p k s", p=128))
                    xtc_by_c[c] = xt

                def prep_b1(c):
                    cs = slice(c * W, (c + 1) * W)
                    xt = xtc_by_c[c]
                    pqk = pprep.tile([128, W], F32, tag="prep", name="pqk")
                    for k in range(4):
                        nc.tensor.matmul(
                            pqk[:], lhsT=wqk_sb[:, k, :], rhs=xt[:, k, :],
                            start=(k == 0), stop=(k == 3))
                    qkbf = pqkbf.tile([128, W], BF16, tag="qkbf", name="qkbf")
                    nc.vector.tensor_copy(qkbf[:], pqk[:])
                    t1 = prt.tile([128, W], BF16, tag="t1", name="t1")
                    t2 = prt.tile([128, W], BF16, tag="t2", name="t2")
                    nc.vector.tensor_mul(t1[:], qkbf[:], cos2[:, cs])
                    nc.vector.tensor_mul(t2[0:32, :], qkbf[32:64, :],
                                         sinneg2[0:32, cs])
                    nc.vector.tensor_mul(t2[32:64, :], qkbf[0:32, :],
                                         sinneg2[32:64, cs])
                    nc.vector.tensor_mul(t2[64:96, :], qkbf[96:128, :],
                                         sinneg2[64:96, cs])
                    nc.vector.tensor_mul(t2[96:128, :], qkbf[64:96, :],
                                         sinneg2[96:128, cs])
                    nc.vector.tensor_add(qt[:, cs], t1[0:64, :], t2[0:64, :])
                    nc.vector.tensor_add(kt[:, cs], t1[64:128, :], t2[64:128, :])

                def prep_b2(c):
                    xt = xtc_by_c.pop(c)
                    pv = pprep.tile([128, W], F32, tag="prep", name="pv")
                    for si in range(4):
                        for k in range(4):
                            nc.tensor.matmul(
                                pv[:, si * HD:(si + 1) * HD],
                                lhsT=xt[:, k, si * 128:(si + 1) * 128],
                                rhs=wv_sb[:, k, :],
                                start=(k == 0), stop=(k == 3))
                    nc.vector.tensor_copy(
                        vext[:].rearrange("p (t c) -> p t c", c=65)[
                            :, 4 * c:4 * c + 4, 0:HD],
                        pv[:, 0:256].rearrange("p (t c) -> p t c", c=HD))

                def attn_groups(sup, otp, glo, ghi):
                    npairs = (sup + 1) * 4
                    q0 = sup * W
                    for g in range(glo, ghi):
                        sp = psc.tile([128, 1024], F32, tag="sc", name="sp")
                        offs = []
                        for p in range(2):
                            kp = g * 2 + p
                            off = max(0, kp * 128 - q0)
                            offs.append(off)
                            nc.tensor.matmul(
                                sp[:, p * W + off:(p + 1) * W],
                                lhsT=kt[:, kp * 128:(kp + 1) * 128],
                                rhs=qt[:, q0 + off:q0 + W],
                                start=True, stop=True)
                        ptile = ppt.tile([128, 1024], BF16, tag="pt",
                                         name="ptile")
                        if offs[0] == 0 and offs[1] == 0:
                            nc.scalar.activation(ptile[:], sp[:], Exp,
                                                 scale=SCALE)
                        else:
                            for p in range(2):
                                o = p * W + offs[p]
                                nc.scalar.activation(
                                    ptile[:, o:(p + 1) * W],
                                    sp[:, o:(p + 1) * W], Exp, scale=SCALE)
                        for p in range(2):
                            kp = g * 2 + p
                            if kp >= sup * 4:
                                o = p * W + offs[p]
                                nc.gpsimd.affine_select(
                                    out=ptile[:, o:o + 128],
                                    in_=ptile[:, o:o + 128],
                                    pattern=[[1, 128]],
                                    compare_op=mybir.AluOpType.is_ge, fill=0.0,
                                    base=0, channel_multiplier=-1)
                        for p in range(2):
                            kp = g * 2 + p
                            off = offs[p]
                            nc.tensor.matmul(
                                otp[:, off:W],
                                lhsT=vext[:, kp * 65:kp * 65 + 65],
                                rhs=ptile[:, p * W + off:(p + 1) * W],
                                start=(kp == 0), stop=(kp == npairs - 1))

                def attn_tail(sup, otp):
                    qs = slice(sup * W, (sup + 1) * W)
                    ot65 = pot.tile([65, W], BF16, tag="ot65", name="ot65")
                    nc.vector.tensor_copy(ot65[:], otp[:])
                    nc.gpsimd.dma_start(rowsums[0:1, qs], ot65[64:65, :])
                    for oi in range(4):
                        po = ppo.tile([128, W], F32, tag="po", name="po")
                        nc.tensor.matmul(
                            po[:], lhsT=woh_sb[:, oi * 128:(oi + 1) * 128],
                            rhs=ot65[0:64, :], start=True, stop=True)
                        posb = pposb.tile([128, W], BF16, tag="posb",
                                          name="posb")
                        nc.vector.tensor_copy(posb[:], po[:])
                        eng = nc.sync if oi % 2 == 0 else nc.gpsimd
                        eng.dma_start(
                            partialT[oi * 128:(oi + 1) * 128, qs], posb[:])

                prep_a(0)
                prep_b1(0)
                prep_b2(0)
                prep_a(1)
                prep_b1(1)
                prep_b2(1)
                for sup in range(NSUP):
                    ngroups = (sup + 1) * 2
                    otp = psot.tile([65, W], F32, tag="otp", name="otp")
                    nxt = sup + 2
                    if nxt < NSUP:
                        attn_groups(sup, otp, 0, max(1, ngroups // 2))
                        prep_a(nxt)
                        attn_groups(sup, otp, max(1, ngroups // 2), ngroups)
                        prep_b1(nxt)
                        prep_b2(nxt)
                    else:
                        attn_groups(sup, otp, 0, ngroups)
                    attn_tail(sup, otp)
    if hoist:
        _split_matmul_waits(nc)
    return nc


def get_program(S=4096):
    if S not in _PROGRAM_CACHE:
        _PROGRAM_CACHE[S] = build_program(S)
    return _PROGRAM_CACHE[S]


def make_in_maps(hidden_states, Wq, Wk, Wv, Wo, cos, sin, num_registers, S):
    """Host-side packing: transpose X, slice per-head weights, fold the
    rotate_half sign into the sin table, build full-length transposed cos/sin
    tables (identity rotation for the register tokens)."""
    nr = int(num_registers)
    X = np.asarray(hidden_states, dtype=np.float32).reshape(S, HIDDEN)
    Wq = np.asarray(Wq, dtype=np.float32)
    Wk = np.asarray(Wk, dtype=np.float32)
    Wv = np.asarray(Wv, dtype=np.float32)
    Wo = np.asarray(Wo, dtype=np.float32)
    cos = np.asarray(cos, dtype=np.float32)
    sin = np.asarray(sin, dtype=np.float32)

    cos_full = np.ones((S, HD), np.float32)
    sin_full = np.zeros((S, HD), np.float32)
    if nr < S:
        cos_full[nr:] = cos[:S - nr]
        sin_full[nr:] = sin[:S - nr]
    cosT = np.ascontiguousarray(cos_full.T)
    sinT = np.ascontiguousarray(sin_full.T)
    sinnegT = np.concatenate([-sinT[:HD // 2], sinT[HD // 2:]], axis=0)

    bf = ml_dtypes.bfloat16
    xT = np.ascontiguousarray(X.T).astype(bf)
    cosT = cosT.astype(bf)
    sinnegT = np.ascontiguousarray(sinnegT).astype(bf)

    in_maps = []
    for c in range(NCORES):
        sl = slice(c * HD, (c + 1) * HD)
        wqk = np.ascontiguousarray(
            np.concatenate([Wq[sl], Wk[sl]], axis=0).T).astype(bf)
        wv_h = np.ascontiguousarray(Wv[sl].T).astype(bf)
        woh = np.ascontiguousarray(Wo[:, sl].T).astype(bf)
        in_maps.append({
            "xT": xT, "wqk": wqk, "wv": wv_h, "woh": woh,
            "cosT": cosT, "sinnegT": sinnegT,
        })
    return in_maps


def kernel(hidden_states, Wq, Wk, Wv, Wo, cos, sin, num_registers):
    hidden_states = np.asarray(hidden_states)
    B, S, H = hidden_states.shape
    assert B == 1 and H == HIDDEN
    nc = get_program(S)
    in_maps = make_in_maps(hidden_states, Wq, Wk, Wv, Wo, cos, sin,
                           num_registers, S)
    res = run_bass_kernel_spmd(nc, in_maps, list(range(NCORES)))
    acc = np.zeros((HIDDEN, S), np.float32)
    for c in range(NCORES):
        p = np.asarray(res.results[c]["partialT"]).astype(np.float32)
        z = np.asarray(res.results[c]["rowsums"]).astype(np.float32)
        acc += p / z
    out = np.ascontiguousarray(acc.T).reshape(1, S, HIDDEN)
    return out.astype(np.float32)


# revision 8
# speedup vs baseline: 1.7537x; 1.0343x over previous
"""Trainium2 Bass kernel for causal multi-head attention with RoPE + register tokens.

Problem (nn_Attention_38293928411140):
  B=1, S=4096, HIDDEN=512, 8 heads x head_dim 64, causal SDPA, RoPE applied to
  positions >= num_registers (cos/sin indexed by position - num_registers), fp32.
  out = softmax(causal(QK^T/8)) V followed by a Wo projection.

Sharding: tensor-parallel over heads -- one head per NeuronCore, no collective.
Each core emits an UNNORMALIZED per-head partial of the output projection
(partialT = Wo_h^T . (exp-scores . V)^T, [512, S]) plus the softmax row-sums;
the host divides by the row-sums and adds the 8 partials.

Per-core kernel, all matmuls bf16 (inputs pre-rounded host-side; tolerance is
2e-2 and measured error is ~2e-3):
  - X^T is transposed on the HOST and streamed in bf16, so no PE transposes.
  - Q^T/K^T projection in one [128,512] psum block per 512-chunk; RoPE applied
    on DVE: rotate_half is a partition shuffle (sign folded into the host-built
    sinneg table), all ops bf16 SBUF->SBUF at 4x DVE rate.
  - V projected directly in natural [s, d] orientation (bf16 matmuls are
    1 cycle/row at any width) -- no V transpose; a ones-column appended to V
    makes the attention matmul produce softmax row-sums for free.
  - causal flash attention in transposed orientation: scores^T [k, q] chunks
    on PE, exp on the scalar engine (the only engine with activation hw;
    max-shift skipped -- exact by shift invariance, scores are bounded),
    diagonal chunks compute/exp only the causally live column range and mask
    just the 128-wide boundary block via gpsimd affine_select after exp.
  - per-supertile tail: psum -> bf16 copy (row 64 = row-sums), 4 output-
    projection matmuls (contract dim 64), psum -> bf16 copies, DMA out.
  - chunk prep for c+1 is emitted interleaved with attention supertile c and
    overlaps it across engines (PE/ACT/DVE/Pool/DMA all concurrently busy).

A post-scheduling pass hoists extra semaphore waits onto sequencer no-ops
because this walrus build rejects instructions with more than one sync wait.
"""
import math
import numpy as np
import ml_dtypes

import concourse.bass as bass
import concourse.mybir as mybir
import concourse.tile as tile

from concourse.bass_utils import run_bass_kernel_spmd

F32 = mybir.dt.float32
BF16 = mybir.dt.bfloat16

HIDDEN = 512
NHEADS = 8
HD = 64
NCORES = 8
SCALE = 1.0 / math.sqrt(HD)

_PROGRAM_CACHE = {}

_HOIST_TYPES = {"InstMatmult", "InstDrain", "InstDMACopy"}


def _split_matmul_waits(nc):
    """Walrus's CoreV3 codegen rejects instructions carrying more than one sync
    wait ('Too many sync wait commands', e.g. Matmult LW_STRUCT and Drain).
    Hoist all but one wait onto same-engine sequencer no-ops inserted right
    before the instruction -- semantically identical (the sequencer satisfies
    the waits in program order before issuing it)."""
    import bass_rust
    for f in nc.m.functions:
        for blk in f.blocks:
            out = []
            for inst in blk.instructions:
                si = getattr(inst, "sync_info", None)
                eng = getattr(inst, "engine", None)
                if si is not None and eng is not None and len(si.on_wait) > 1:
                    waits = list(si.on_wait)
                    for k, w in enumerate(waits[:-1]):
                        nop = bass_rust.InstNoOp(
                            name=f"{inst.name}-hw{k}",
                            engine=eng,
                            text_hint="hoisted-wait",
                            sync_info=mybir.SyncInfo(on_wait=[w], on_update=[]),
                        )
                        out.append(nop)
                    inst.sync_info = mybir.SyncInfo(
                        on_wait=[waits[-1]], on_update=list(si.on_update))
                out.append(inst)
            blk.instructions = out


def build_program(S=4096, hoist=True, repeat=1, mock_cc=False, hw_loop=0,
                  fast_mm=True):
    """Build the SPMD Bass program (same NEFF on all 8 cores, no collectives).

    Fused causal pipeline: supertile `sup` of the attention only needs Q/K/V
    chunks <= sup, so chunk prep for sup+1 is emitted interleaved with
    attention supertile sup and overlaps it across engines."""
    assert S % 512 == 0
    W = 512                      # q-supertile width == s-chunk width
    NSUP = S // W
    NST = S // 128

    nc = bass.Bass("TRN2", target_bir_lowering=False, debug=False,
                   num_devices=NCORES)

    xT = nc.dram_tensor("xT", [HIDDEN, S], BF16, kind="ExternalInput").ap()
    wqk = nc.dram_tensor("wqk", [HIDDEN, 2 * HD], BF16, kind="ExternalInput").ap()
    wv = nc.dram_tensor("wv", [HIDDEN, HD], BF16, kind="ExternalInput").ap()
    woh = nc.dram_tensor("woh", [HD, HIDDEN], BF16, kind="ExternalInput").ap()
    cosT = nc.dram_tensor("cosT", [HD, S], BF16, kind="ExternalInput").ap()
    sinnegT = nc.dram_tensor("sinnegT", [HD, S], BF16, kind="ExternalInput").ap()
    partialT = nc.dram_tensor("partialT", [HIDDEN, S], BF16,
                              kind="ExternalOutput").ap()
    rowsums = nc.dram_tensor("rowsums", [1, S], BF16,
                             kind="ExternalOutput").ap()

    Exp = mybir.ActivationFunctionType.Exp

    with tile.TileContext(nc) as tc:
      with tc.tile_pool(name="persist", bufs=1) as pp:
        wqk_sb = pp.tile([128, 4, 128], BF16, tag="wqk")
        wv_sb = pp.tile([128, 4, HD], BF16, tag="wv")
        woh_sb = pp.tile([HD, HIDDEN], BF16, tag="woh")
        cos2 = pp.tile([128, S], BF16, tag="cos2")
        sinneg2 = pp.tile([128, S], BF16, tag="sinneg2")
        qt = pp.tile([HD, S], BF16, tag="qt")        # roped Q^T [d, s]
        kt = pp.tile([HD, S], BF16, tag="kt")        # roped K^T [d, s]
        vext = pp.tile([128, NST * 65], BF16, tag="vext")  # V tiles + ones col

        nc.gpsimd.dma_start(wqk_sb[:],
                            wqk.rearrange("(k p) c -> p k c", p=128))
        nc.gpsimd.dma_start(wv_sb[:],
                            wv.rearrange("(k p) c -> p k c", p=128))
        nc.gpsimd.dma_start(woh_sb[:], woh)
        # ones columns of vext (data columns are overwritten per chunk)
        nc.gpsimd.memset(
            vext[:].rearrange("p (t c) -> p t c", c=65)[:, :, 64:65], 1.0)

        import contextlib
        loop_cm = tc.For_i(0, hw_loop, 1) if hw_loop else contextlib.nullcontext()
        with loop_cm:
          for _rep in range(repeat):
            with tc.tile_pool(name="xin", bufs=3) as pxin, \
                 tc.tile_pool(name="qkbf", bufs=2) as pqkbf, \
                 tc.tile_pool(name="rope", bufs=2) as prt, \
                 tc.tile_pool(name="pt", bufs=3) as ppt, \
                 tc.tile_pool(name="otb", bufs=2) as pot, \
                 tc.tile_pool(name="pob", bufs=3) as pposb, \
                 tc.tile_pool(name="psc", bufs=2, space="PSUM") as psc, \
                 tc.tile_pool(name="pprep", bufs=2, space="PSUM") as pprep, \
                 tc.tile_pool(name="psot", bufs=1, space="PSUM") as psot, \
                 tc.tile_pool(name="ppo", bufs=1, space="PSUM") as ppo:

                xtc_by_c = {}

                def prep_a(c):
                    cs = slice(c * W, (c + 1) * W)
                    xt = pxin.tile([128, 4, W], BF16, tag="xin", name="xt")
                    nc.sync.dma_start(
                        xt[:],
                        xT[:, cs].rearrange("(k p) s -> p k s", p=128))
                    if _rep == 0:
                        # stream rope tables chunk-by-chunk into both 64-row
                        # halves of the duplicated tables
                        nc.sync.dma_start(cos2[0:HD, cs], cosT[:, cs])
                        nc.sync.dma_start(cos2[HD:128, cs], cosT[:, cs])
                        nc.sync.dma_start(sinneg2[0:HD, cs], sinnegT[:, cs])
                        nc.sync.dma_start(sinneg2[HD:128, cs], sinnegT[:, cs])
                    xtc_by_c[c] = xt

                def prep_b1(c):
                    cs = slice(c * W, (c + 1) * W)
                    xt = xtc_by_c[c]
                    pqk = pprep.tile([128, W], F32, tag="prep", name="pqk")
                    for k in range(4):
                        nc.tensor.matmul(
                            pqk[:], lhsT=wqk_sb[:, k, :], rhs=xt[:, k, :],
                            start=(k == 0), stop=(k == 3))
                    qkbf = pqkbf.tile([128, W], BF16, tag="qkbf", name="qkbf")
                    nc.vector.tensor_copy(qkbf[:], pqk[:])
                    t1 = prt.tile([128, W], BF16, tag="t1", name="t1")
                    t2 = prt.tile([128, W], BF16, tag="t2", name="t2")
                    nc.vector.tensor_mul(t1[:], qkbf[:], cos2[:, cs])
                    nc.vector.tensor_mul(t2[0:32, :], qkbf[32:64, :],
                                         sinneg2[0:32, cs])
                    nc.vector.tensor_mul(t2[32:64, :], qkbf[0:32, :],
                                         sinneg2[32:64, cs])
                    nc.vector.tensor_mul(t2[64:96, :], qkbf[96:128, :],
                                         sinneg2[64:96, cs])
                    nc.vector.tensor_mul(t2[96:128, :], qkbf[64:96, :],
                                         sinneg2[96:128, cs])
                    nc.vector.tensor_add(qt[:, cs], t1[0:64, :], t2[0:64, :])
                    nc.vector.tensor_add(kt[:, cs], t1[64:128, :], t2[64:128, :])

                def prep_b2(c):
                    xt = xtc_by_c.pop(c)
                    pv = pprep.tile([128, W], F32, tag="prep", name="pv")
                    for si in range(4):
                        for k in range(4):
                            nc.tensor.matmul(
                                pv[:, si * HD:(si + 1) * HD],
                                lhsT=xt[:, k, si * 128:(si + 1) * 128],
                                rhs=wv_sb[:, k, :],
                                start=(k == 0), stop=(k == 3))
                    nc.vector.tensor_copy(
                        vext[:].rearrange("p (t c) -> p t c", c=65)[
                            :, 4 * c:4 * c + 4, 0:HD],
                        pv[:, 0:256].rearrange("p (t c) -> p t c", c=HD))

                def emit_scores(sup, g):
                    q0 = sup * W
                    sp = psc.tile([128, 1024], F32, tag="sc", name="sp")
                    offs = []
                    for p in range(2):
                        kp = g * 2 + p
                        off = max(0, kp * 128 - q0)
                        offs.append(off)
                        nc.tensor.matmul(
                            sp[:, p * W + off:(p + 1) * W],
                            lhsT=kt[:, kp * 128:(kp + 1) * 128],
                            rhs=qt[:, q0 + off:q0 + W],
                            start=True, stop=True)
                    return sp, offs

                def emit_expav(sup, otp, g, sp, offs):
                    npairs = (sup + 1) * 4
                    ptile = ppt.tile([128, 1024], BF16, tag="pt",
                                     name="ptile")
                    if offs[0] == 0 and offs[1] == 0:
                        nc.scalar.activation(ptile[:], sp[:], Exp,
                                             scale=SCALE)
                    else:
                        for p in range(2):
                            o = p * W + offs[p]
                            nc.scalar.activation(
                                ptile[:, o:(p + 1) * W],
                                sp[:, o:(p + 1) * W], Exp, scale=SCALE)
                    for p in range(2):
                        kp = g * 2 + p
                        if kp >= sup * 4:
                            o = p * W + offs[p]
                            nc.gpsimd.affine_select(
                                out=ptile[:, o:o + 128],
                                in_=ptile[:, o:o + 128],
                                pattern=[[1, 128]],
                                compare_op=mybir.AluOpType.is_ge, fill=0.0,
                                base=0, channel_multiplier=-1)
                    for p in range(2):
                        kp = g * 2 + p
                        off = offs[p]
                        nc.tensor.matmul(
                            otp[:, off:W],
                            lhsT=vext[:, kp * 65:kp * 65 + 65],
                            rhs=ptile[:, p * W + off:(p + 1) * W],
                            start=(kp == 0), stop=(kp == npairs - 1))

                def attn_sup(sup, otp, hooks):
                    """Emit all groups of a supertile, scores one group ahead
                    of exp+AV so the scalar engine never waits on PE.
                    hooks[g] are prep closures emitted before group g."""
                    ngroups = (sup + 1) * 2
                    pend = emit_scores(sup, 0)
                    for g in range(ngroups):
                        for fn in hooks.get(g, ()):
                            fn()
                        sp, offs = pend
                        if g + 1 < ngroups:
                            pend = emit_scores(sup, g + 1)
                        emit_expav(sup, otp, g, sp, offs)
                    for fn in hooks.get(ngroups, ()):
                        fn()

                def attn_tail(sup, otp):
                    last = sup == NSUP - 1
                    qs = slice(sup * W, (sup + 1) * W)
                    ot65 = pot.tile([65, W], BF16, tag="ot65", name="ot65")
                    nc.vector.tensor_copy(ot65[:], otp[:])
                    nc.gpsimd.dma_start(rowsums[0:1, qs], ot65[64:65, :])
                    for oi in range(4):
                        if last:
                            # scores psum pool is free now; ping-pong po
                            # through it and split copies ACT/DVE to shorten
                            # the serial tail
                            po = psc.tile([128, 1024], F32, tag="sc",
                                          name="po")[:, 0:W]
                        else:
                            po = ppo.tile([128, W], F32, tag="po", name="po")
                        nc.tensor.matmul(
                            po[:], lhsT=woh_sb[:, oi * 128:(oi + 1) * 128],
                            rhs=ot65[0:64, :], start=True, stop=True)
                        posb = pposb.tile([128, W], BF16, tag="posb",
                                          name="posb")
                        if last and oi % 2 == 0:
                            nc.scalar.copy(posb[:], po[:])
                        else:
                            nc.vector.tensor_copy(posb[:], po[:])
                        eng = nc.sync if oi % 2 == 0 else nc.gpsimd
                        eng.dma_start(
                            partialT[oi * 128:(oi + 1) * 128, qs], posb[:])

                prep_a(0)
                prep_b1(0)
                prep_b2(0)
                prep_a(1)
                prep_b1(1)
                prep_b2(1)
                for sup in range(NSUP):
                    ngroups = (sup + 1) * 2
                    otp = psot.tile([65, W], F32, tag="otp", name="otp")
                    nxt = sup + 2
                    hooks = {}
                    if nxt < NSUP:
                        third = max(1, ngroups // 3)
                        hooks.setdefault(third, []).append(
                            lambda c=nxt: prep_a(c))
                        hooks.setdefault(min(2 * third, ngroups - 1),
                                         []).append(lambda c=nxt: prep_b1(c))
                        hooks.setdefault(ngroups, []).append(
                            lambda c=nxt: prep_b2(c))
                    attn_sup(sup, otp, hooks)
                    attn_tail(sup, otp)
    if hoist:
        _split_matmul_waits(nc)
    return nc


def get_program(S=4096):
    if S not in _PROGRAM_CACHE:
        _PROGRAM_CACHE[S] = build_program(S)
    return _PROGRAM_CACHE[S]


def make_in_maps(hidden_states, Wq, Wk, Wv, Wo, cos, sin, num_registers, S):
    """Host-side packing: transpose X, slice per-head weights, fold the
    rotate_half sign into the sin table, build full-length transposed cos/sin
    tables (identity rotation for the register tokens)."""
    nr = int(num_registers)
    X = np.asarray(hidden_states, dtype=np.float32).reshape(S, HIDDEN)
    Wq = np.asarray(Wq, dtype=np.float32)
    Wk = np.asarray(Wk, dtype=np.float32)
    Wv = np.asarray(Wv, dtype=np.float32)
    Wo = np.asarray(Wo, dtype=np.float32)
    cos = np.asarray(cos, dtype=np.float32)
    sin = np.asarray(sin, dtype=np.float32)

    cos_full = np.ones((S, HD), np.float32)
    sin_full = np.zeros((S, HD), np.float32)
    if nr < S:
        cos_full[nr:] = cos[:S - nr]
        sin_full[nr:] = sin[:S - nr]
    cosT = np.ascontiguousarray(cos_full.T)
    sinT = np.ascontiguousarray(sin_full.T)
    sinnegT = np.concatenate([-sinT[:HD // 2], sinT[HD // 2:]], axis=0)

    bf = ml_dtypes.bfloat16
    xT = np.ascontiguousarray(X.T).astype(bf)
    cosT = cosT.astype(bf)
    sinnegT = np.ascontiguousarray(sinnegT).astype(bf)

    in_maps = []
    for c in range(NCORES):
        sl = slice(c * HD, (c + 1) * HD)
        wqk = np.ascontiguousarray(
            np.concatenate([Wq[sl], Wk[sl]], axis=0).T).astype(bf)
        wv_h = np.ascontiguousarray(Wv[sl].T).astype(bf)
        woh = np.ascontiguousarray(Wo[:, sl].T).astype(bf)
        in_maps.append({
            "xT": xT, "wqk": wqk, "wv": wv_h, "woh": woh,
            "cosT": cosT, "sinnegT": sinnegT,
        })
    return in_maps


def kernel(hidden_states, Wq, Wk, Wv, Wo, cos, sin, num_registers):
    hidden_states = np.asarray(hidden_states)
    B, S, H = hidden_states.shape
    assert B == 1 and H == HIDDEN
    nc = get_program(S)
    in_maps = make_in_maps(hidden_states, Wq, Wk, Wv, Wo, cos, sin,
                           num_registers, S)
    res = run_bass_kernel_spmd(nc, in_maps, list(range(NCORES)))
    acc = np.zeros((HIDDEN, S), np.float32)
    for c in range(NCORES):
        p = np.asarray(res.results[c]["partialT"]).astype(np.float32)
        z = np.asarray(res.results[c]["rowsums"]).astype(np.float32)
        acc += p / z
    out = np.ascontiguousarray(acc.T).reshape(1, S, HIDDEN)
    return out.astype(np.float32)
